# revision 25
# baseline (speedup 1.0000x reference)
"""Trainium2 Bass kernel for nn_Decimate: 129-tap polyphase FIR decimation by q=4.

The reference's blocked-FFT conv is mathematically a strided valid correlation
    y[b, i] = sum_{j=0}^{128} x_ext[b, 4i + j] * k[j],   i in [0, 262144)
where x_ext = [reflect_64(x), x, zeros_64]  (length 1048704 = 128 * 8193).

Device scheme (per NeuronCore, 2 batch rows each across 8 cores):
  - x_ext is chunked into 128-element chunks, deinterleaved into 4 phase
    planes  plane_r[c', :] = chunk[4c' + r]  in bf16 (rel-err budget is
    2e-2; bf16 in / bf16 out measures ~2.7e-3), transposed to
    partition-major X[p, c'] on host, so the device does only large plain
    DMAs (one [128 x 528] bf16 load per (row, slab, plane)).
  - Toeplitz weights W_s[p, i0] = k[128 s + p - 4 i0] (5 shifts) in bf16.
    W_s is nonzero only on an i0 band: s=0:[0,32) 1:[0,64) 2:[32,96)
    3:[64,128) 4:[96,128) — moving columns restricted to bands.
  - Tensor engine, signal stationary / weights moving:
        O[c', i0] = sum_s X_s[:, c'block].T @ W_s
    PSUM-accumulated over 5 banded matmuls (one full-width start to zero
    the PSUM region, then banded accumulation).
  - O is copied PSUM->SBUF as bf16 (vector/scalar alternating) and stored
    with a fully contiguous per-(row,slab) DMA; the host un-permutes and
    upcasts to fp32.
"""

import numpy as np
import ml_dtypes

import concourse.bacc as bacc
import concourse.mybir as mybir
import concourse.tile as tile
from concourse.bass_utils import run_bass_kernel_spmd
from concourse.vector_clock import ScopedClock


class _LeanTile(tile.TileContext):
    """TileContext whose epilogue uses sem-only all-engine barriers.

    Keeps the full shutdown protocol (drain with global-clock waits, barrier,
    semaphore clears, barrier) so NEFF re-execution stays safe, but replaces
    the two drain-based multi_engine_barrier calls with the cheaper
    sem-inc/wait barrier flavor.
    """

    def _drain_and_barrier(self, tick_clock, wait_clock):
        drain_inst = self.nc.sync.drain()
        wait_clock.add_sem_waits(
            drain_inst.ins, ScopedClock({None: tick_clock.global_clock}))
        self.nc.all_engine_barrier(sem_only=True)
        popped = self.nc._tile_sem_poison_stack.pop()
        assert popped is self._sem_poison
        self.nc.clear_and_free_semaphores(
            list(self.sems.allocated().values()))
        self.nc.all_engine_barrier(sem_only=True)


bf16 = ml_dtypes.bfloat16

# Problem constants (hardcoded per harness contract)
T = 1048576
NTAP = 129
Q = 4
PAD = 64
ROWS = 16
N_CORES = 8
ROWS_PER_CORE = ROWS // N_CORES          # 2
OUT = T // Q                             # 262144 outputs per row
CBLK = 128                               # elements per input chunk
NCH_P = 8196                             # chunks, padded to multiple of 4
PLANE_COLS = NCH_P // 4                  # 2049
PLANE_ROWS = 2064                        # padded plane length
NCPRIME = OUT // CBLK                    # 2048 output chunks per row
SLAB_C = 512                             # output-chunk columns per slab
SLAB_W = 516                             # slab width incl. halo (513 used)
N_SLABS = NCPRIME // SLAB_C              # 4 slab groups per row
BLOCKS_PER_SLAB = SLAB_C // 128          # 4
NPLANE = 4                               # 4 phase planes (bf16 only)

# i0-bands where W_s is nonzero.  The first matmul of a slab carries
# start=True, which claims and zeroes the whole 2 KB PSUM zero-region
# (bank), so every matmul can run at its natural band width and all 20
# matmuls of a slab form a single accumulation group in one bank.
# (s, lo, hi, wbase): W_s[:, lo:hi] is stored packed at w[:, wbase:...]
COMBO = [(1, 0, 64, 0), (0, 0, 32, 64), (2, 32, 96, 96),
         (3, 64, 128, 160), (4, 96, 128, 224)]
WCOLS = 256                              # packed band columns

_PROGRAM = None


def _build_weights(k):
    """W[s, p, i0] = k[128 s + p - 4 i0] masked to j in [0, 128]."""
    W = np.zeros((5, 128, 128), dtype=np.float32)
    p = np.arange(128)[:, None]
    i0 = np.arange(128)[None, :]
    for s in range(5):
        j = 128 * s + p - 4 * i0
        m = (j >= 0) & (j <= 128)
        W[s][m] = k[j[m]]
    return W


def _build_planes(x):
    """x: [B, T] fp32 -> phase planes [B, 4, PLANE_ROWS, 128] fp32."""
    B = x.shape[0]
    xe = np.zeros((B, NCH_P * CBLK), dtype=np.float32)
    xe[:, PAD:PAD + T] = x
    xe[:, :PAD] = x[:, 1:PAD + 1][:, ::-1]
    ch = xe.reshape(B, PLANE_COLS, 4, CBLK)
    planes = np.zeros((B, 4, PLANE_ROWS, CBLK), dtype=np.float32)
    planes[:, :, :PLANE_COLS, :] = ch.transpose(0, 2, 1, 3)
    return planes


def _build_program():
    """Build the per-core Bass/Tile program (same NEFF on all 8 cores)."""
    # Bacc (not raw Bass): its compile() splits multi-wait sync lists into
    # InstEventSemaphore chains — TRN2 allows only 1 wait per instruction.
    nc = bacc.Bacc(None)
    f32 = mybir.dt.float32
    b16 = mybir.dt.bfloat16

    # xs[row, slab, p, plane, col] — each (row, slab) is ONE contiguous
    # [128, 4x528] bf16 DMA (4224 B per partition): DMA issue costs ~600 ns
    # of sequencer time each, so fewer/bigger issues win, and 4 KB-contiguous
    # partition lines keep the per-DMA-engine packet rate high.
    xs = nc.declare_dram_parameter(
        "xs", [ROWS_PER_CORE, N_SLABS, CBLK, NPLANE, SLAB_W], b16,
        isOutput=False)
    # w[p, j]: the 5 Toeplitz shifts, band columns only, packed per COMBO
    w = nc.declare_dram_parameter("w", [CBLK, WCOLS], b16, isOutput=False)
    # y[row, slab, c', bl, i0] bf16 — matches the stage tile layout so the
    # store DMA is fully contiguous; host un-permutes to [row, out].
    y = nc.declare_dram_parameter(
        "y", [ROWS_PER_CORE, N_SLABS, CBLK, BLOCKS_PER_SLAB, CBLK], b16,
        isOutput=True)

    with _LeanTile(nc) as tc:
        with (
            tc.tile_pool(name="wpool", bufs=1) as wpool,
            tc.tile_pool(name="xpool", bufs=8) as xpool,
            tc.tile_pool(name="opool", bufs=8) as opool,
            tc.tile_pool(name="psum", bufs=8, space="PSUM") as psum_pool,
        ):
            w_t = wpool.tile([CBLK, WCOLS], b16, tag="w")
            nc.scalar.dma_start(out=w_t[:], in_=w[:])

            # Issue ALL 8 slab loads up-front, alternating between the two
            # HWDGE rings (sync / scalar, each 4 deep) so both rings sit
            # fully queued with no refill stalls and no compute instruction
            # ever delays an input issue in program order.
            xt = []
            for row in range(ROWS_PER_CORE):
                for g in range(N_SLABS):
                    t = xpool.tile([CBLK, NPLANE, SLAB_W], b16, tag="xs")
                    eng = nc.sync if (2 * row + g) % 2 == 0 else nc.scalar
                    eng.dma_start(out=t[:], in_=xs[row, g])
                    xt.append(t)

            for row in range(ROWS_PER_CORE):
                for g in range(N_SLABS):
                    t = xt[N_SLABS * row + g]
                    stage = opool.tile([CBLK, BLOCKS_PER_SLAB * CBLK], b16,
                                       tag="stage")
                    # One accumulation group for the whole slab: a full-bank
                    # [128, 512] PSUM tile, 20 banded matmuls, one 512-wide
                    # PSUM->SBUF bf16 copy.
                    O = psum_pool.tile([CBLK, BLOCKS_PER_SLAB * CBLK], f32,
                                       tag="O")
                    nmm = BLOCKS_PER_SLAB * len(COMBO)
                    i = 0
                    for bl in range(BLOCKS_PER_SLAB):
                        for s, lo, hi, wb in COMBO:
                            r, off = s % 4, s // 4
                            c0 = 128 * bl + off
                            nc.tensor.matmul(
                                O[:, 128 * bl + lo:128 * bl + hi],
                                t[:, r, c0:c0 + 128],
                                w_t[:, wb:wb + (hi - lo)],
                                start=(i == 0), stop=(i == nmm - 1))
                            i += 1
                    if (2 * row + g) % 2 == 0:
                        nc.vector.tensor_copy(stage[:], O[:])
                    else:
                        nc.scalar.copy(stage[:], O[:])
                    # stores go through gpsimd's SWDGE queue so both HWDGE
                    # rings stay dedicated to input slabs; the last store
                    # uses sync's (by then idle) HW ring for its lower
                    # issue-to-transfer latency
                    if row == ROWS_PER_CORE - 1 and g == N_SLABS - 1:
                        nc.sync.dma_start(out=y[row, g], in_=stage[:])
                    else:
                        nc.gpsimd.dma_start(out=y[row, g], in_=stage[:])
    nc.finalize()
    return nc


def _get_program():
    global _PROGRAM
    if _PROGRAM is None:
        _PROGRAM = _build_program()
    return _PROGRAM


def _prepare_in_maps(x, k):
    planes = _build_planes(np.ascontiguousarray(x, dtype=np.float32))
    ph = planes.astype(bf16)
    # host-side transpose to partition-major [B, 4, p, c]
    ph = np.ascontiguousarray(ph.swapaxes(2, 3))

    # pack [B, slab, p, plane, col]
    B = x.shape[0]
    xsv = np.zeros((B, N_SLABS, CBLK, NPLANE, SLAB_W), dtype=bf16)
    for g in range(N_SLABS):
        sl = slice(SLAB_C * g, SLAB_C * g + SLAB_W)
        for r in range(NPLANE):
            xsv[:, g, :, r, :] = ph[:, r, :, sl]

    W = _build_weights(np.asarray(k, dtype=np.float32))
    # weight layout [p, packed band cols] per COMBO
    w_t = np.zeros((CBLK, WCOLS), dtype=bf16)
    for s, lo, hi, wb in COMBO:
        w_t[:, wb:wb + (hi - lo)] = W[s][:, lo:hi].astype(bf16)

    in_maps = []
    for c in range(N_CORES):
        sl = slice(c * ROWS_PER_CORE, (c + 1) * ROWS_PER_CORE)
        in_maps.append({
            "xs": np.ascontiguousarray(xsv[sl]),
            "w": w_t,
        })
    return in_maps


def _run(x, k, trace=False):
    nc = _get_program()
    in_maps = _prepare_in_maps(x, k)
    res = run_bass_kernel_spmd(nc, in_maps, list(range(N_CORES)), trace=trace)
    outs = []
    for r in res.results:
        # y[row, g, c', bl, i0] -> out[row, 512 g + 128 bl + c', i0]
        yv = np.asarray(r["y"]).astype(np.float32)
        yv = yv.transpose(0, 1, 3, 2, 4).reshape(ROWS_PER_CORE, OUT)
        outs.append(yv)
    out = np.concatenate(outs, axis=0).reshape(ROWS, OUT)
    return out, res


def kernel(x, kernel, q):
    assert int(q) == Q and x.shape == (ROWS, T) and kernel.shape == (NTAP,)
    out, _ = _run(np.asarray(x), np.asarray(kernel), trace=False)
    return out


def kernel_traced(x, kernel, q):
    """Like kernel() but returns (out, BassKernelResults) with HW profile."""
    out, res = _run(np.asarray(x), np.asarray(kernel), trace=True)
    return out, res


# revision 26
# speedup vs baseline: 1.0192x; 1.0192x over previous
"""Trainium2 Bass kernel for nn_Decimate: 129-tap polyphase FIR decimation by q=4.

The reference's blocked-FFT conv is mathematically a strided valid correlation
    y[b, i] = sum_{j=0}^{128} x_ext[b, 4i + j] * k[j],   i in [0, 262144)
where x_ext = [reflect_64(x), x, zeros_64]  (length 1048704 = 128 * 8193).

Device scheme (per NeuronCore, 2 batch rows each across 8 cores):
  - x_ext is chunked into 128-element chunks, deinterleaved into 4 phase
    planes  plane_r[c', :] = chunk[4c' + r]  in bf16 (rel-err budget is
    2e-2; bf16 in / bf16 out measures ~2.7e-3), transposed to
    partition-major X[p, c'] on host, so the device does only large plain
    DMAs (one [128 x 528] bf16 load per (row, slab, plane)).
  - Toeplitz weights W_s[p, i0] = k[128 s + p - 4 i0] (5 shifts) in bf16.
    W_s is nonzero only on an i0 band: s=0:[0,32) 1:[0,64) 2:[32,96)
    3:[64,128) 4:[96,128) — moving columns restricted to bands.
  - Tensor engine, signal stationary / weights moving:
        O[c', i0] = sum_s X_s[:, c'block].T @ W_s
    PSUM-accumulated over 5 banded matmuls (one full-width start to zero
    the PSUM region, then banded accumulation).
  - O is copied PSUM->SBUF as bf16 (vector/scalar alternating) and stored
    with a fully contiguous per-(row,slab) DMA; the host un-permutes and
    upcasts to fp32.
"""

import numpy as np
import ml_dtypes

import concourse.bacc as bacc
import concourse.mybir as mybir
import concourse.tile as tile
from concourse.bass_utils import run_bass_kernel_spmd
from concourse.vector_clock import ScopedClock


class _LeanTile(tile.TileContext):
    """TileContext whose epilogue uses sem-only all-engine barriers.

    Keeps the full shutdown protocol (drain with global-clock waits, barrier,
    semaphore clears, barrier) so NEFF re-execution stays safe, but replaces
    the two drain-based multi_engine_barrier calls with the cheaper
    sem-inc/wait barrier flavor.
    """

    def _drain_and_barrier(self, tick_clock, wait_clock):
        drain_inst = self.nc.sync.drain()
        wait_clock.add_sem_waits(
            drain_inst.ins, ScopedClock({None: tick_clock.global_clock}))
        self.nc.all_engine_barrier(sem_only=True)
        popped = self.nc._tile_sem_poison_stack.pop()
        assert popped is self._sem_poison
        self.nc.clear_and_free_semaphores(
            list(self.sems.allocated().values()))
        self.nc.all_engine_barrier(sem_only=True)


bf16 = ml_dtypes.bfloat16

# Problem constants (hardcoded per harness contract)
T = 1048576
NTAP = 129
Q = 4
PAD = 64
ROWS = 16
N_CORES = 8
ROWS_PER_CORE = ROWS // N_CORES          # 2
OUT = T // Q                             # 262144 outputs per row
CBLK = 128                               # elements per input chunk
NCH_P = 8196                             # chunks, padded to multiple of 4
PLANE_COLS = NCH_P // 4                  # 2049
PLANE_ROWS = 2064                        # padded plane length
NCPRIME = OUT // CBLK                    # 2048 output chunks per row
SLAB_C = 512                             # output-chunk columns per slab
SLAB_W = 516                             # slab width incl. halo (513 used)
N_SLABS = NCPRIME // SLAB_C              # 4 slab groups per row
BLOCKS_PER_SLAB = SLAB_C // 128          # 4
NPLANE = 4                               # 4 phase planes (bf16 only)

# i0-bands where W_s is nonzero.  The first matmul of a slab carries
# start=True, which claims and zeroes the whole 2 KB PSUM zero-region
# (bank), so every matmul can run at its natural band width and all 20
# matmuls of a slab form a single accumulation group in one bank.
# (s, lo, hi, wbase): W_s[:, lo:hi] is stored packed at w[:, wbase:...]
COMBO = [(1, 0, 64, 0), (0, 0, 32, 64), (2, 32, 96, 96),
         (3, 64, 128, 160), (4, 96, 128, 224)]
WCOLS = 256                              # packed band columns

_PROGRAM = None


def _build_weights(k):
    """W[s, p, i0] = k[128 s + p - 4 i0] masked to j in [0, 128]."""
    W = np.zeros((5, 128, 128), dtype=np.float32)
    p = np.arange(128)[:, None]
    i0 = np.arange(128)[None, :]
    for s in range(5):
        j = 128 * s + p - 4 * i0
        m = (j >= 0) & (j <= 128)
        W[s][m] = k[j[m]]
    return W


def _build_planes(x):
    """x: [B, T] fp32 -> phase planes [B, 4, PLANE_ROWS, 128] fp32."""
    B = x.shape[0]
    xe = np.zeros((B, NCH_P * CBLK), dtype=np.float32)
    xe[:, PAD:PAD + T] = x
    xe[:, :PAD] = x[:, 1:PAD + 1][:, ::-1]
    ch = xe.reshape(B, PLANE_COLS, 4, CBLK)
    planes = np.zeros((B, 4, PLANE_ROWS, CBLK), dtype=np.float32)
    planes[:, :, :PLANE_COLS, :] = ch.transpose(0, 2, 1, 3)
    return planes


def _build_program():
    """Build the per-core Bass/Tile program (same NEFF on all 8 cores)."""
    # Bacc (not raw Bass): its compile() splits multi-wait sync lists into
    # InstEventSemaphore chains — TRN2 allows only 1 wait per instruction.
    nc = bacc.Bacc(None)
    f32 = mybir.dt.float32
    b16 = mybir.dt.bfloat16

    # xs[row, slab, p, plane, col] — each (row, slab) is ONE contiguous
    # [128, 4x528] bf16 DMA (4224 B per partition): DMA issue costs ~600 ns
    # of sequencer time each, so fewer/bigger issues win, and 4 KB-contiguous
    # partition lines keep the per-DMA-engine packet rate high.
    xs = nc.declare_dram_parameter(
        "xs", [ROWS_PER_CORE, N_SLABS, CBLK, NPLANE, SLAB_W], b16,
        isOutput=False)
    # w[p, j]: the 5 Toeplitz shifts, band columns only, packed per COMBO
    w = nc.declare_dram_parameter("w", [CBLK, WCOLS], b16, isOutput=False)
    # y[row, slab, c', bl, i0] bf16 — matches the stage tile layout so the
    # store DMA is fully contiguous; host un-permutes to [row, out].
    y = nc.declare_dram_parameter(
        "y", [ROWS_PER_CORE, N_SLABS, CBLK, BLOCKS_PER_SLAB, CBLK], b16,
        isOutput=True)

    with _LeanTile(nc) as tc:
        with (
            tc.tile_pool(name="wpool", bufs=1) as wpool,
            tc.tile_pool(name="xpool", bufs=8) as xpool,
            tc.tile_pool(name="opool", bufs=8) as opool,
            tc.tile_pool(name="psum", bufs=8, space="PSUM") as psum_pool,
        ):
            w_t = wpool.tile([CBLK, WCOLS], b16, tag="w")
            nc.scalar.dma_start(out=w_t[:], in_=w[:])

            # Issue ALL 8 slab loads up-front, alternating between the two
            # HWDGE rings (sync / scalar, each 4 deep) so both rings sit
            # fully queued with no refill stalls and no compute instruction
            # ever delays an input issue in program order.
            xt = []
            for row in range(ROWS_PER_CORE):
                for g in range(N_SLABS):
                    t = xpool.tile([CBLK, NPLANE, SLAB_W], b16, tag="xs")
                    eng = nc.sync if (2 * row + g) % 2 == 0 else nc.scalar
                    eng.dma_start(out=t[:], in_=xs[row, g])
                    xt.append(t)

            for row in range(ROWS_PER_CORE):
                for g in range(N_SLABS):
                    t = xt[N_SLABS * row + g]
                    stage = opool.tile([CBLK, BLOCKS_PER_SLAB * CBLK], b16,
                                       tag="stage")
                    # One accumulation group for the whole slab: a full-bank
                    # [128, 512] PSUM tile, 20 banded matmuls, one 512-wide
                    # PSUM->SBUF bf16 copy.
                    O = psum_pool.tile([CBLK, BLOCKS_PER_SLAB * CBLK], f32,
                                       tag="O")
                    nmm = BLOCKS_PER_SLAB * len(COMBO)
                    i = 0
                    for bl in range(BLOCKS_PER_SLAB):
                        for s, lo, hi, wb in COMBO:
                            r, off = s % 4, s // 4
                            c0 = 128 * bl + off
                            nc.tensor.matmul(
                                O[:, 128 * bl + lo:128 * bl + hi],
                                t[:, r, c0:c0 + 128],
                                w_t[:, wb:wb + (hi - lo)],
                                start=(i == 0), stop=(i == nmm - 1))
                            i += 1
                    if row == ROWS_PER_CORE - 1 and g == N_SLABS - 1:
                        # tail trim: halve the final copy across both engines
                        # (they run concurrently) and store the halves on
                        # separate HW rings
                        yv = y[row, g].rearrange("c b i -> c (b i)")
                        nc.vector.tensor_copy(stage[:, :256], O[:, :256])
                        nc.scalar.copy(stage[:, 256:], O[:, 256:])
                        nc.sync.dma_start(out=yv[:, :256], in_=stage[:, :256])
                        nc.scalar.dma_start(out=yv[:, 256:],
                                            in_=stage[:, 256:])
                    else:
                        if (2 * row + g) % 2 == 0:
                            nc.vector.tensor_copy(stage[:], O[:])
                        else:
                            nc.scalar.copy(stage[:], O[:])
                        # stores go on sync's ring, which is idle once its
                        # four input issues are out
                        nc.sync.dma_start(out=y[row, g], in_=stage[:])
    nc.finalize()
    return nc


def _get_program():
    global _PROGRAM
    if _PROGRAM is None:
        _PROGRAM = _build_program()
    return _PROGRAM


def _prepare_in_maps(x, k):
    planes = _build_planes(np.ascontiguousarray(x, dtype=np.float32))
    ph = planes.astype(bf16)
    # host-side transpose to partition-major [B, 4, p, c]
    ph = np.ascontiguousarray(ph.swapaxes(2, 3))

    # pack [B, slab, p, plane, col]
    B = x.shape[0]
    xsv = np.zeros((B, N_SLABS, CBLK, NPLANE, SLAB_W), dtype=bf16)
    for g in range(N_SLABS):
        sl = slice(SLAB_C * g, SLAB_C * g + SLAB_W)
        for r in range(NPLANE):
            xsv[:, g, :, r, :] = ph[:, r, :, sl]

    W = _build_weights(np.asarray(k, dtype=np.float32))
    # weight layout [p, packed band cols] per COMBO
    w_t = np.zeros((CBLK, WCOLS), dtype=bf16)
    for s, lo, hi, wb in COMBO:
        w_t[:, wb:wb + (hi - lo)] = W[s][:, lo:hi].astype(bf16)

    in_maps = []
    for c in range(N_CORES):
        sl = slice(c * ROWS_PER_CORE, (c + 1) * ROWS_PER_CORE)
        in_maps.append({
            "xs": np.ascontiguousarray(xsv[sl]),
            "w": w_t,
        })
    return in_maps


def _run(x, k, trace=False):
    nc = _get_program()
    in_maps = _prepare_in_maps(x, k)
    res = run_bass_kernel_spmd(nc, in_maps, list(range(N_CORES)), trace=trace)
    outs = []
    for r in res.results:
        # y[row, g, c', bl, i0] -> out[row, 512 g + 128 bl + c', i0]
        yv = np.asarray(r["y"]).astype(np.float32)
        yv = yv.transpose(0, 1, 3, 2, 4).reshape(ROWS_PER_CORE, OUT)
        outs.append(yv)
    out = np.concatenate(outs, axis=0).reshape(ROWS, OUT)
    return out, res


def kernel(x, kernel, q):
    assert int(q) == Q and x.shape == (ROWS, T) and kernel.shape == (NTAP,)
    out, _ = _run(np.asarray(x), np.asarray(kernel), trace=False)
    return out


def kernel_traced(x, kernel, q):
    """Like kernel() but returns (out, BassKernelResults) with HW profile."""
    out, res = _run(np.asarray(x), np.asarray(kernel), trace=True)
    return out, res
